# revision 68
# baseline (speedup 1.0000x reference)
"""AdaAtt attention kernel for 8 Trainium2 NeuronCores.

Pure data-parallel: batch dim B=2048 sharded 256 rows per core; weights
replicated. Each core computes, for its shard (R=A=1024, G=49):

    fr  = relu(fake_region @ Wf1.T + bf1)
    fre = fr @ Wf2.T + bf2
    hl  = tanh(h_out @ Wh1.T + bh1)
    he  = hl @ Wh2.T + bh2
    hA[g]  = tanh(embed[g] + he)        embed = [fre, conv_feat_embed]
    scores[g] = hA[g] . Wa              (+ ba: dropped - softmax shift-invariant)
    PI = softmax(scores)
    visAtt = sum_g PI[g] * img[g]       img = [fr, conv_feat]
    out = tanh((visAtt + hl) @ W2h.T + b2h)

Layouts are all natural [batch-partition, feature-free]. Big tensors are
converted to bf16 on the host. The Wa-dot uses scalar_tensor_tensor with
accum_out (scores land as [128b, g] directly); visAtt runs on the
TensorEngine as diag(PI_g) matmuls accumulating in PSUM fp32.
"""
import numpy as np
import ml_dtypes
from contextlib import ExitStack

import concourse.bass as bass
import concourse.mybir as mybir
import concourse.tile as tile
from concourse import bacc
from concourse.bass_utils import run_bass_kernel_spmd
from concourse.masks import make_identity

BF16 = mybir.dt.bfloat16
F32 = mybir.dt.float32
AF = mybir.ActivationFunctionType
ALU = mybir.AluOpType

N_CORES = 8
B, R, A, G = 2048, 1024, 1024, 49
BS = B // N_CORES          # 256 rows per core
NB = BS // 128             # 2 partition-tiles per core
KC = R // 128              # 8 k-chunks
GB = 4                     # g's per streamed block

_CACHED_NC = None


def _build():
    nc = bacc.Bacc("TRN2", target_bir_lowering=False, debug=False,
                   num_devices=N_CORES)

    xh_d = nc.dram_tensor("xh", [R, BS], BF16, kind="ExternalInput").ap()  # h_out.T
    xf_d = nc.dram_tensor("xf", [R, BS], BF16, kind="ExternalInput").ap()  # fake_region.T
    cfe_d = nc.dram_tensor("cfe", [BS, G, A], BF16, kind="ExternalInput").ap()
    cf_d = nc.dram_tensor("cf", [BS, G, R], BF16, kind="ExternalInput").ap()
    w_d = {}
    b_d = {}
    for name in ("wf1t", "wf2t", "wh1t", "wh2t", "w2ht"):
        w_d[name] = nc.dram_tensor(name, [R, R], BF16, kind="ExternalInput").ap()
    for name in ("bf1", "bf2", "bh1", "bh2", "b2h"):
        b_d[name] = nc.dram_tensor(name, [1, R], BF16, kind="ExternalInput").ap()
    wa_d = nc.dram_tensor("wa", [1, A], BF16, kind="ExternalInput").ap()
    out_d = nc.dram_tensor("out", [BS, R], F32, kind="ExternalOutput").ap()

    with ExitStack() as ctx:
        tc = ctx.enter_context(tile.TileContext(nc))
        consts = ctx.enter_context(tc.tile_pool(name="consts", bufs=1))
        wpool = ctx.enter_context(tc.tile_pool(name="wpool", bufs=1))
        acts = ctx.enter_context(tc.tile_pool(name="acts", bufs=1))
        xt_pool = ctx.enter_context(tc.tile_pool(name="xt", bufs=2))
        embp = ctx.enter_context(tc.tile_pool(name="embp", bufs=4))
        cvp = ctx.enter_context(tc.tile_pool(name="cvp", bufs=7))
        ha_pool = ctx.enter_context(tc.tile_pool(name="ha", bufs=3))
        diag_pool = ctx.enter_context(tc.tile_pool(name="diag", bufs=3))
        small = ctx.enter_context(tc.tile_pool(name="small", bufs=4))
        tp_psum = ctx.enter_context(tc.tile_pool(name="tp_ps", bufs=2, space="PSUM"))
        y_psum = ctx.enter_context(tc.tile_pool(name="y_ps", bufs=2, space="PSUM"))
        vis_psum = ctx.enter_context(tc.tile_pool(name="vis_ps", bufs=1, space="PSUM"))
        add_psum = ctx.enter_context(tc.tile_pool(name="add_ps", bufs=2, space="PSUM"))

        # ---- constants / inputs (DMA order = priority: acts first so the
        # first dense starts early; w2ht loaded late) ----
        ident = consts.tile([128, 128], BF16)
        make_identity(nc, ident)
        ones1 = consts.tile([1, 128], BF16)
        nc.vector.memset(ones1, 1.0)

        # pre-transposed inputs: [k-part, kchunk, btile, b] ready as matmul lhsT
        xhT_sb = acts.tile([128, KC, NB, 128], BF16)
        nc.sync.dma_start(
            out=xhT_sb,
            in_=xh_d.rearrange("(c p) (nb b) -> p c nb b", p=128, b=128))
        xfT_sb = acts.tile([128, KC, NB, 128], BF16)
        nc.sync.dma_start(
            out=xfT_sb,
            in_=xf_d.rearrange("(c p) (nb b) -> p c nb b", p=128, b=128))

        w_sb = {}
        b_sb = {}

        def load_w(name):
            # one tile + DMA per k-chunk: matmuls on early chunks start while
            # the rest of the weight still streams in. Chunks share a 24-slot
            # ring (5 weights x 8 chunks cycle through; at most ~16 live).
            src = w_d[name].rearrange("(c p) n -> p c n", p=128)
            chunks = []
            for k in range(KC):
                wk = wpool.tile([128, R], BF16, name=f"{name}_k{k}",
                                tag="wk", bufs=24)
                nc.sync.dma_start(out=wk, in_=src[:, k, :])
                chunks.append(wk)
            w_sb[name] = chunks

        def load_b(name):
            b_sb[name] = wpool.tile([1, R], BF16, name=name + "_sb")
            nc.sync.dma_start(out=b_sb[name], in_=b_d[name])

        for name in ("bh1", "bh2", "bf1", "bf2", "b2h"):
            load_b(name)
        wa_bcast = consts.tile([128, A], BF16)
        wa_rep = bass.AP(tensor=wa_d.tensor, offset=wa_d.offset,
                         ap=[[0, 128]] + list(wa_d.ap[1:]))
        nc.gpsimd.dma_start(out=wa_bcast, in_=wa_rep)
        for name in ("wh1t", "wh2t"):
            load_w(name)

        fr_sb = acts.tile([128, NB, R], BF16)
        fre_sb = acts.tile([128, NB, R], BF16)
        hl_sb = acts.tile([128, NB, R], BF16)
        he_sb = acts.tile([128, NB, R], BF16)
        sum_sb = acts.tile([128, NB, R], BF16)
        out_sb = acts.tile([128, NB, R], F32)
        scores = acts.tile([128, NB, 64], F32)
        exps = acts.tile([128, NB, 64], F32)
        pi = acts.tile([128, NB, 64], F32)

        def dense_nb(nb, w, b, func, o_sb, x_sb=None, xT=None):
            """o[:, nb, :] = func(x[:, nb, :] @ W.T + bias) for one btile.

            Either xT (pre-transposed [128, KC, 128] lhsT chunks) or x_sb
            (natural layout, transposed here via the PE) must be given.
            """
            if xT is None:
                xT = xt_pool.tile([128, KC, 128], BF16, tag="xT",
                                  name="xT_dense")
                for k in range(KC):
                    tp = tp_psum.tile([128, 128], BF16, tag="tp")
                    nc.tensor.transpose(
                        tp, x_sb[:, nb, k * 128:(k + 1) * 128], ident)
                    nc.scalar.copy(out=xT[:, k, :], in_=tp)
            for n in range(R // 512):
                yp = y_psum.tile([128, 512], F32, tag="yp")
                for k in range(KC):
                    nc.tensor.matmul(
                        yp, lhsT=xT[:, k, :],
                        rhs=w[k][:, n * 512:(n + 1) * 512],
                        start=(k == 0), stop=False)
                nc.tensor.matmul(yp, lhsT=ones1,
                                 rhs=b[:, n * 512:(n + 1) * 512],
                                 start=False, stop=True)
                nc.scalar.activation(
                    out=o_sb[:, nb, n * 512:(n + 1) * 512], in_=yp, func=func)

        # ---- phase 1a: the he-chain (gates phase 2) ----
        for nb in range(NB):
            dense_nb(nb, w_sb["wh1t"], b_sb["bh1"], AF.Tanh, hl_sb,
                     xT=xhT_sb[:, :, nb, :])
            dense_nb(nb, w_sb["wh2t"], b_sb["bh2"], AF.Copy, he_sb, x_sb=hl_sb)

        # g-blocks over the DRAM conv tensors (g' = ref_g - 1)
        blocks = [list(range(s, min(s + GB, G))) for s in range(0, G, GB)]

        # ---- phase 2 + softmax + phase 3, per batch-tile ----
        vps = []
        for nb in range(NB):
            bsl = slice(nb * 128, (nb + 1) * 128)

            def wa_dot(col, th):
                st = ha_pool.tile([128, A], BF16, tag="st", name="st", bufs=2)
                nc.vector.scalar_tensor_tensor(
                    out=st, in0=th, scalar=1.0, in1=wa_bcast,
                    op0=ALU.bypass, op1=ALU.mult,
                    accum_out=scores[:, nb, col:col + 1])

            def score_col_dve(col, src):
                # add on DVE, tanh on ScalarE, Wa-dot fused on DVE
                hs = ha_pool.tile([128, A], BF16, tag="st", name="hs", bufs=2)
                nc.vector.tensor_add(hs, src, he_sb[:, nb, :])
                th = ha_pool.tile([128, A], BF16, tag="th", name="th")
                nc.scalar.activation(out=th, in_=hs, func=AF.Tanh)
                wa_dot(col, th)

            def score_col_pe(col, src):
                # add on the TensorEngine (identity-matmul PSUM accumulate)
                th = ha_pool.tile([128, A], BF16, tag="th", name="th")
                for h in range(2):
                    hsl = slice(h * 512, (h + 1) * 512)
                    ap = add_psum.tile([128, 512], F32, tag="ap", name="ap")
                    nc.tensor.matmul(ap, lhsT=ident, rhs=src[:, hsl],
                                     start=True, stop=False)
                    nc.tensor.matmul(ap, lhsT=ident, rhs=he_sb[:, nb, hsl],
                                     start=False, stop=True)
                    nc.scalar.activation(out=th[:, hsl], in_=ap, func=AF.Tanh)
                wa_dot(col, th)

            # phase 2: embed stream -> scores columns 1..G; the "+he" adds
            # alternate between DVE and the TensorEngine to balance load
            for bi, blk in enumerate(blocks):
                emb = embp.tile([128, GB, A], BF16, tag="emb")
                if nb == 0 and bi < 4:
                    # DMA queues round-robin at packet granularity, so an
                    # ungated embed stream steals bandwidth from the
                    # critical phase-1 weight loads. A 1-element copy makes
                    # this DMA order after the last wh2t chunk.
                    nc.gpsimd.tensor_copy(out=emb[:, 0, 0:1],
                                          in_=w_sb["wh2t"][KC - 1][:, 0:1])
                nc.sync.dma_start(out=emb[:, :len(blk), :],
                                  in_=cfe_d[bsl, blk[0]:blk[-1] + 1, :])
                for j, g in enumerate(blk):
                    if (g + nb) % 2 == 0:
                        score_col_dve(g + 1, emb[:, j, :])
                    else:
                        score_col_pe(g + 1, emb[:, j, :])
                if nb == 0 and bi == 2:
                    # phase 1b part 1: fr (needed for visAtt g=0, ~mid-way)
                    load_w("wf1t")
                    for nb2 in range(NB):
                        dense_nb(nb2, w_sb["wf1t"], b_sb["bf1"], AF.Relu,
                                 fr_sb, xT=xfT_sb[:, :, nb2, :])
                if nb == 0 and bi == 5:
                    # phase 1b part 2: fre (needed for scores col 0, later)
                    load_w("wf2t")
                    for nb2 in range(NB):
                        dense_nb(nb2, w_sb["wf2t"], b_sb["bf2"], AF.Copy,
                                 fre_sb, x_sb=fr_sb)

            if nb == 0:
                # w2ht queues behind the whole nb0 embed stream: it lands
                # in the mid-kernel DMA lull instead of the early crunch
                load_w("w2ht")

            # scores column 0 comes from fre (+ he on DVE)
            score_col_dve(0, fre_sb[:, nb, :])

            # softmax over the G+1 scores
            mx = small.tile([128, 1], F32, tag="mx")
            nc.vector.tensor_reduce(mx, scores[:, nb, 0:G + 1],
                                    axis=mybir.AxisListType.X, op=ALU.max)
            nmx = small.tile([128, 1], F32, tag="nmx")
            nc.vector.tensor_scalar_mul(nmx, mx, -1.0)
            sume = small.tile([128, 1], F32, tag="sume")
            nc.scalar.activation(out=exps[:, nb, 0:G + 1],
                                 in_=scores[:, nb, 0:G + 1],
                                 func=AF.Exp, bias=nmx, scale=1.0,
                                 accum_out=sume)
            rs = small.tile([128, 1], F32, tag="rs")
            nc.vector.reciprocal(rs, sume)
            nc.vector.tensor_scalar_mul(pi[:, nb, 0:G + 1],
                                        exps[:, nb, 0:G + 1], rs)

            # phase 3: conv stream -> visAtt (diag(PI_g) matmuls into PSUM)
            vp = vis_psum.tile([128, R], F32, tag="vis", name="vp")
            vps.append(vp)
            d0 = diag_pool.tile([128, 128], BF16, tag="diag")
            nc.vector.tensor_scalar_mul(d0, ident, pi[:, nb, 0:1])
            for h in range(2):
                nc.tensor.matmul(vp[:, h * 512:(h + 1) * 512], lhsT=d0,
                                 rhs=fr_sb[:, nb, h * 512:(h + 1) * 512],
                                 start=True, stop=False)
            for ci, blk in enumerate(blocks):
                # the last nb1 blocks borrow slots from the embed pool
                # (free by then): they prefetch into the late-phase-2 DMA
                # idle window instead of waiting for cv-slot recycling
                if nb == 1 and ci >= 9:
                    cv = embp.tile([128, GB, R], BF16, tag="emb", name="cv2")
                else:
                    cv = cvp.tile([128, GB, R], BF16, tag="cv", name="cv")
                # SWDGE queue: independent of the sync-ring embed stream, so
                # conv blocks prefetch throughout phase 2
                if nb == 0:
                    # keep the conv prefetch off the phase-1 weight loads
                    nc.gpsimd.tensor_copy(out=cv[:, 0, 0:1],
                                          in_=w_sb["wh2t"][KC - 1][:, 0:1])
                nc.gpsimd.dma_start(out=cv[:, :len(blk), :],
                                    in_=cf_d[bsl, blk[0]:blk[-1] + 1, :])
                for j, g in enumerate(blk):
                    dg = diag_pool.tile([128, 128], BF16, tag="diag")
                    if nb == 1:
                        # tail: ScalarE is idle here, DVE is the pacer
                        nc.scalar.mul(dg, ident, pi[:, nb, g + 1:g + 2])
                    else:
                        nc.vector.tensor_scalar_mul(dg, ident,
                                                    pi[:, nb, g + 1:g + 2])
                    lastg = (g == G - 1)
                    for h in range(2):
                        nc.tensor.matmul(vp[:, h * 512:(h + 1) * 512],
                                         lhsT=dg,
                                         rhs=cv[:, j, h * 512:(h + 1) * 512],
                                         start=False, stop=lastg)
            # visAtt + hl, then the final dense + store for this btile
            nc.vector.tensor_add(sum_sb[:, nb, :], vp, hl_sb[:, nb, :])
            dense_nb(nb, w_sb["w2ht"], b_sb["b2h"], AF.Tanh, out_sb,
                     x_sb=sum_sb)
            nc.sync.dma_start(
                out=out_d.rearrange("(nb p) n -> p nb n", p=128)[:, nb, :],
                in_=out_sb[:, nb, :])

    nc.compile()
    return nc


def _get_nc():
    global _CACHED_NC
    if _CACHED_NC is None:
        _CACHED_NC = _build()
    return _CACHED_NC


def _prep_inputs(h_out, fake_region, conv_feat, conv_feat_embed,
                 Wf1, bf1, Wf2, bf2, Wh1, bh1, Wh2, bh2, Wa, ba, W2h, b2h):
    bf = ml_dtypes.bfloat16
    xh = np.ascontiguousarray(np.asarray(h_out, np.float32).T).astype(bf)
    xf = np.ascontiguousarray(np.asarray(fake_region, np.float32).T).astype(bf)
    cf = np.asarray(conv_feat, np.float32).astype(bf)
    cfe = np.asarray(conv_feat_embed, np.float32).astype(bf)
    shared = {
        "wf1t": np.ascontiguousarray(np.asarray(Wf1, np.float32).T).astype(bf),
        "wf2t": np.ascontiguousarray(np.asarray(Wf2, np.float32).T).astype(bf),
        "wh1t": np.ascontiguousarray(np.asarray(Wh1, np.float32).T).astype(bf),
        "wh2t": np.ascontiguousarray(np.asarray(Wh2, np.float32).T).astype(bf),
        "w2ht": np.ascontiguousarray(np.asarray(W2h, np.float32).T).astype(bf),
        "bf1": np.asarray(bf1, np.float32).reshape(1, R).astype(bf),
        "bf2": np.asarray(bf2, np.float32).reshape(1, R).astype(bf),
        "bh1": np.asarray(bh1, np.float32).reshape(1, R).astype(bf),
        "bh2": np.asarray(bh2, np.float32).reshape(1, R).astype(bf),
        "b2h": np.asarray(b2h, np.float32).reshape(1, R).astype(bf),
        "wa": np.asarray(Wa, np.float32).reshape(1, A).astype(bf),
    }
    in_maps = []
    for i in range(N_CORES):
        s = slice(i * BS, (i + 1) * BS)
        m = dict(shared)
        m["xh"] = np.ascontiguousarray(xh[:, s])
        m["xf"] = np.ascontiguousarray(xf[:, s])
        m["cf"] = np.ascontiguousarray(cf[s])
        m["cfe"] = np.ascontiguousarray(cfe[s])
        in_maps.append(m)
    return in_maps


def kernel(**inputs):
    nc = _get_nc()
    in_maps = _prep_inputs(**inputs)
    res = run_bass_kernel_spmd(nc, in_maps, core_ids=list(range(N_CORES)))
    return np.concatenate([res.results[i]["out"] for i in range(N_CORES)], axis=0)


def run_traced(**inputs):
    """Like kernel() but with NTFF tracing; returns (out, BassKernelResults)."""
    nc = _get_nc()
    in_maps = _prep_inputs(**inputs)
    res = run_bass_kernel_spmd(nc, in_maps, core_ids=list(range(N_CORES)),
                               trace=True)
    out = np.concatenate([res.results[i]["out"] for i in range(N_CORES)], axis=0)
    return out, res


# revision 70
# speedup vs baseline: 1.1170x; 1.1170x over previous
"""AdaAtt attention kernel for 8 Trainium2 NeuronCores.

Pure data-parallel: batch dim B=2048 sharded 256 rows per core; weights
replicated. Each core computes, for its shard (R=A=1024, G=49):

    fr  = relu(fake_region @ Wf1.T + bf1)
    fre = fr @ Wf2.T + bf2
    hl  = tanh(h_out @ Wh1.T + bh1)
    he  = hl @ Wh2.T + bh2
    hA[g]  = tanh(embed[g] + he)        embed = [fre, conv_feat_embed]
    scores[g] = hA[g] . Wa              (+ ba: dropped - softmax shift-invariant)
    PI = softmax(scores)
    visAtt = sum_g PI[g] * img[g]       img = [fr, conv_feat]
    out = tanh((visAtt + hl) @ W2h.T + b2h)

Layouts are all natural [batch-partition, feature-free]. Big tensors are
converted to bf16 on the host. The Wa-dot uses scalar_tensor_tensor with
accum_out (scores land as [128b, g] directly); visAtt runs on the
TensorEngine as diag(PI_g) matmuls accumulating in PSUM fp32.
"""
import numpy as np
import ml_dtypes
from contextlib import ExitStack

import concourse.bass as bass
import concourse.mybir as mybir
import concourse.tile as tile
from concourse import bacc
from concourse.bass_utils import run_bass_kernel_spmd
from concourse.masks import make_identity

BF16 = mybir.dt.bfloat16
F32 = mybir.dt.float32
AF = mybir.ActivationFunctionType
ALU = mybir.AluOpType

N_CORES = 8
B, R, A, G = 2048, 1024, 1024, 49
BS = B // N_CORES          # 256 rows per core
NB = BS // 128             # 2 partition-tiles per core
KC = R // 128              # 8 k-chunks
GB = 4                     # g's per streamed block

_CACHED_NC = None


def _build():
    nc = bacc.Bacc("TRN2", target_bir_lowering=False, debug=False,
                   num_devices=N_CORES)

    xh_d = nc.dram_tensor("xh", [R, BS], BF16, kind="ExternalInput").ap()  # h_out.T
    xf_d = nc.dram_tensor("xf", [R, BS], BF16, kind="ExternalInput").ap()  # fake_region.T
    cfe_d = nc.dram_tensor("cfe", [BS, G, A], BF16, kind="ExternalInput").ap()
    cf_d = nc.dram_tensor("cf", [BS, G, R], BF16, kind="ExternalInput").ap()
    w_d = {}
    b_d = {}
    for name in ("wf1t", "wf2t", "wh1t", "wh2t", "w2ht"):
        w_d[name] = nc.dram_tensor(name, [R, R], BF16, kind="ExternalInput").ap()
    for name in ("bf1", "bf2", "bh1", "bh2", "b2h"):
        b_d[name] = nc.dram_tensor(name, [1, R], BF16, kind="ExternalInput").ap()
    wa_d = nc.dram_tensor("wa", [1, A], BF16, kind="ExternalInput").ap()
    out_d = nc.dram_tensor("out", [BS, R], F32, kind="ExternalOutput").ap()

    with ExitStack() as ctx:
        tc = ctx.enter_context(tile.TileContext(nc))
        consts = ctx.enter_context(tc.tile_pool(name="consts", bufs=1))
        wpool = ctx.enter_context(tc.tile_pool(name="wpool", bufs=1))
        acts = ctx.enter_context(tc.tile_pool(name="acts", bufs=1))
        xt_pool = ctx.enter_context(tc.tile_pool(name="xt", bufs=2))
        embp = ctx.enter_context(tc.tile_pool(name="embp", bufs=4))
        cvp = ctx.enter_context(tc.tile_pool(name="cvp", bufs=7))
        ha_pool = ctx.enter_context(tc.tile_pool(name="ha", bufs=3))
        diag_pool = ctx.enter_context(tc.tile_pool(name="diag", bufs=3))
        small = ctx.enter_context(tc.tile_pool(name="small", bufs=4))
        tp_psum = ctx.enter_context(tc.tile_pool(name="tp_ps", bufs=2, space="PSUM"))
        y_psum = ctx.enter_context(tc.tile_pool(name="y_ps", bufs=2, space="PSUM"))
        vis_psum = ctx.enter_context(tc.tile_pool(name="vis_ps", bufs=1, space="PSUM"))
        add_psum = ctx.enter_context(tc.tile_pool(name="add_ps", bufs=2, space="PSUM"))

        # ---- constants / inputs (DMA order = priority: acts first so the
        # first dense starts early; w2ht loaded late) ----
        ident = consts.tile([128, 128], BF16)
        make_identity(nc, ident)
        ones1 = consts.tile([1, 128], BF16)
        nc.vector.memset(ones1, 1.0)

        # pre-transposed inputs: [k-part, kchunk, btile, b] ready as matmul lhsT
        xhT_sb = acts.tile([128, KC, NB, 128], BF16)
        nc.sync.dma_start(
            out=xhT_sb,
            in_=xh_d.rearrange("(c p) (nb b) -> p c nb b", p=128, b=128))
        xfT_sb = acts.tile([128, KC, NB, 128], BF16)
        nc.sync.dma_start(
            out=xfT_sb,
            in_=xf_d.rearrange("(c p) (nb b) -> p c nb b", p=128, b=128))

        w_sb = {}
        b_sb = {}

        def load_w(name):
            # one tile + DMA per k-chunk: matmuls on early chunks start while
            # the rest of the weight still streams in. Chunks share a 24-slot
            # ring (5 weights x 8 chunks cycle through; at most ~16 live).
            src = w_d[name].rearrange("(c p) n -> p c n", p=128)
            chunks = []
            for k in range(KC):
                wk = wpool.tile([128, R], BF16, name=f"{name}_k{k}",
                                tag="wk", bufs=24)
                nc.sync.dma_start(out=wk, in_=src[:, k, :])
                chunks.append(wk)
            w_sb[name] = chunks

        def load_b(name):
            b_sb[name] = wpool.tile([1, R], BF16, name=name + "_sb")
            nc.sync.dma_start(out=b_sb[name], in_=b_d[name])

        for name in ("bh1", "bh2", "bf1", "bf2", "b2h"):
            load_b(name)
        wa_bcast = consts.tile([128, A], BF16)
        wa_rep = bass.AP(tensor=wa_d.tensor, offset=wa_d.offset,
                         ap=[[0, 128]] + list(wa_d.ap[1:]))
        nc.gpsimd.dma_start(out=wa_bcast, in_=wa_rep)
        for name in ("wh1t", "wh2t"):
            load_w(name)

        fr_sb = acts.tile([128, NB, R], BF16)
        fre_sb = acts.tile([128, NB, R], BF16)
        hl_sb = acts.tile([128, NB, R], BF16)
        he_sb = acts.tile([128, NB, R], BF16)
        sum_sb = acts.tile([128, NB, R], BF16)
        out_sb = acts.tile([128, NB, R], F32)
        scores = acts.tile([128, NB, 64], F32)
        exps = acts.tile([128, NB, 64], F32)
        pi = acts.tile([128, NB, 64], F32)

        def dense_nb(nb, w, b, func, o_sb, x_sb=None, xT=None):
            """o[:, nb, :] = func(x[:, nb, :] @ W.T + bias) for one btile.

            Either xT (pre-transposed [128, KC, 128] lhsT chunks) or x_sb
            (natural layout, transposed here via the PE) must be given.
            """
            if xT is None:
                xT = xt_pool.tile([128, KC, 128], BF16, tag="xT",
                                  name="xT_dense")
                for k in range(KC):
                    tp = tp_psum.tile([128, 128], BF16, tag="tp")
                    nc.tensor.transpose(
                        tp, x_sb[:, nb, k * 128:(k + 1) * 128], ident)
                    nc.scalar.copy(out=xT[:, k, :], in_=tp)
            for n in range(R // 512):
                yp = y_psum.tile([128, 512], F32, tag="yp")
                for k in range(KC):
                    nc.tensor.matmul(
                        yp, lhsT=xT[:, k, :],
                        rhs=w[k][:, n * 512:(n + 1) * 512],
                        start=(k == 0), stop=False)
                nc.tensor.matmul(yp, lhsT=ones1,
                                 rhs=b[:, n * 512:(n + 1) * 512],
                                 start=False, stop=True)
                nc.scalar.activation(
                    out=o_sb[:, nb, n * 512:(n + 1) * 512], in_=yp, func=func)

        # ---- phase 1a: the he-chain (gates phase 2) ----
        for nb in range(NB):
            dense_nb(nb, w_sb["wh1t"], b_sb["bh1"], AF.Tanh, hl_sb,
                     xT=xhT_sb[:, :, nb, :])
            dense_nb(nb, w_sb["wh2t"], b_sb["bh2"], AF.Copy, he_sb, x_sb=hl_sb)

        # g-blocks over the DRAM conv tensors (g' = ref_g - 1)
        blocks = [list(range(s, min(s + GB, G))) for s in range(0, G, GB)]

        # ---- phase 2 + softmax + phase 3, per batch-tile ----
        vps = []
        for nb in range(NB):
            bsl = slice(nb * 128, (nb + 1) * 128)

            def wa_dot(col, th):
                st = ha_pool.tile([128, A], BF16, tag="st", name="st", bufs=2)
                nc.vector.scalar_tensor_tensor(
                    out=st, in0=th, scalar=1.0, in1=wa_bcast,
                    op0=ALU.bypass, op1=ALU.mult,
                    accum_out=scores[:, nb, col:col + 1])

            def score_col_dve(col, src):
                # add on DVE, tanh on ScalarE, Wa-dot fused on DVE
                hs = ha_pool.tile([128, A], BF16, tag="st", name="hs", bufs=2)
                nc.vector.tensor_add(hs, src, he_sb[:, nb, :])
                th = ha_pool.tile([128, A], BF16, tag="th", name="th")
                nc.scalar.activation(out=th, in_=hs, func=AF.Tanh)
                wa_dot(col, th)

            def score_col_pe(col, src):
                # add on the TensorEngine (identity-matmul PSUM accumulate)
                th = ha_pool.tile([128, A], BF16, tag="th", name="th")
                for h in range(2):
                    hsl = slice(h * 512, (h + 1) * 512)
                    ap = add_psum.tile([128, 512], F32, tag="ap", name="ap")
                    nc.tensor.matmul(ap, lhsT=ident, rhs=src[:, hsl],
                                     start=True, stop=False)
                    nc.tensor.matmul(ap, lhsT=ident, rhs=he_sb[:, nb, hsl],
                                     start=False, stop=True)
                    nc.scalar.activation(out=th[:, hsl], in_=ap, func=AF.Tanh)
                wa_dot(col, th)

            if nb == 1:
                # fre is long ready: do scores col 0 first so softmax-nb1
                # waits only on the last streamed block, not this column
                score_col_dve(0, fre_sb[:, nb, :])

            # phase 2: embed stream -> scores columns 1..G; the "+he" adds
            # alternate between DVE and the TensorEngine to balance load
            for bi, blk in enumerate(blocks):
                emb = embp.tile([128, GB, A], BF16, tag="emb")
                if nb == 0 and bi < 4:
                    # DMA queues round-robin at packet granularity, so an
                    # ungated embed stream steals bandwidth from the
                    # critical phase-1 weight loads. A 1-element copy makes
                    # this DMA order after the last wh2t chunk.
                    nc.gpsimd.tensor_copy(out=emb[:, 0, 0:1],
                                          in_=w_sb["wh2t"][KC - 1][:, 0:1])
                nc.sync.dma_start(out=emb[:, :len(blk), :],
                                  in_=cfe_d[bsl, blk[0]:blk[-1] + 1, :])
                for j, g in enumerate(blk):
                    if (g + nb) % 2 == 0:
                        score_col_dve(g + 1, emb[:, j, :])
                    else:
                        score_col_pe(g + 1, emb[:, j, :])
                if nb == 0 and bi == 2:
                    # phase 1b: the fr-chain — needed mid-stream (fre:
                    # scores col 0; fr: visAtt g=0); weight DMAs queue
                    # behind the first embed blocks
                    for name in ("wf1t", "wf2t"):
                        load_w(name)
                    for nb2 in range(NB):
                        dense_nb(nb2, w_sb["wf1t"], b_sb["bf1"], AF.Relu,
                                 fr_sb, xT=xfT_sb[:, :, nb2, :])
                        dense_nb(nb2, w_sb["wf2t"], b_sb["bf2"], AF.Copy,
                                 fre_sb, x_sb=fr_sb)
                if nb == 0 and bi == 8:
                    load_w("w2ht")

            if nb == 0:
                # scores column 0 comes from fre (+ he on DVE)
                score_col_dve(0, fre_sb[:, nb, :])

            # softmax over the G+1 scores
            mx = small.tile([128, 1], F32, tag="mx")
            nc.vector.tensor_reduce(mx, scores[:, nb, 0:G + 1],
                                    axis=mybir.AxisListType.X, op=ALU.max)
            nmx = small.tile([128, 1], F32, tag="nmx")
            nc.vector.tensor_scalar_mul(nmx, mx, -1.0)
            sume = small.tile([128, 1], F32, tag="sume")
            nc.scalar.activation(out=exps[:, nb, 0:G + 1],
                                 in_=scores[:, nb, 0:G + 1],
                                 func=AF.Exp, bias=nmx, scale=1.0,
                                 accum_out=sume)
            rs = small.tile([128, 1], F32, tag="rs")
            nc.vector.reciprocal(rs, sume)
            nc.vector.tensor_scalar_mul(pi[:, nb, 0:G + 1],
                                        exps[:, nb, 0:G + 1], rs)

            # phase 3: conv stream -> visAtt (diag(PI_g) matmuls into PSUM)
            vp = vis_psum.tile([128, R], F32, tag="vis", name="vp")
            vps.append(vp)
            d0 = diag_pool.tile([128, 128], BF16, tag="diag")
            nc.vector.tensor_scalar_mul(d0, ident, pi[:, nb, 0:1])
            for h in range(2):
                nc.tensor.matmul(vp[:, h * 512:(h + 1) * 512], lhsT=d0,
                                 rhs=fr_sb[:, nb, h * 512:(h + 1) * 512],
                                 start=True, stop=False)
            for ci, blk in enumerate(blocks):
                # the last nb1 blocks borrow slots from the embed pool
                # (free by then): they prefetch into the late-phase-2 DMA
                # idle window instead of waiting for cv-slot recycling
                if nb == 1 and ci >= 9:
                    cv = embp.tile([128, GB, R], BF16, tag="emb", name="cv2")
                else:
                    cv = cvp.tile([128, GB, R], BF16, tag="cv", name="cv")
                # SWDGE queue: independent of the sync-ring embed stream, so
                # conv blocks prefetch throughout phase 2
                if nb == 0:
                    # keep the conv prefetch off the phase-1 weight loads
                    nc.gpsimd.tensor_copy(out=cv[:, 0, 0:1],
                                          in_=w_sb["wh2t"][KC - 1][:, 0:1])
                nc.gpsimd.dma_start(out=cv[:, :len(blk), :],
                                    in_=cf_d[bsl, blk[0]:blk[-1] + 1, :])
                for j, g in enumerate(blk):
                    dg = diag_pool.tile([128, 128], BF16, tag="diag")
                    if nb == 1:
                        # tail: ScalarE is idle here, DVE is the pacer
                        nc.scalar.mul(dg, ident, pi[:, nb, g + 1:g + 2])
                    else:
                        nc.vector.tensor_scalar_mul(dg, ident,
                                                    pi[:, nb, g + 1:g + 2])
                    lastg = (g == G - 1)
                    for h in range(2):
                        nc.tensor.matmul(vp[:, h * 512:(h + 1) * 512],
                                         lhsT=dg,
                                         rhs=cv[:, j, h * 512:(h + 1) * 512],
                                         start=False, stop=lastg)
            # visAtt + hl, then the final dense + store for this btile
            nc.vector.tensor_add(sum_sb[:, nb, :], vp, hl_sb[:, nb, :])
            dense_nb(nb, w_sb["w2ht"], b_sb["b2h"], AF.Tanh, out_sb,
                     x_sb=sum_sb)
            nc.sync.dma_start(
                out=out_d.rearrange("(nb p) n -> p nb n", p=128)[:, nb, :],
                in_=out_sb[:, nb, :])

    nc.compile()
    return nc


def _get_nc():
    global _CACHED_NC
    if _CACHED_NC is None:
        _CACHED_NC = _build()
    return _CACHED_NC


def _prep_inputs(h_out, fake_region, conv_feat, conv_feat_embed,
                 Wf1, bf1, Wf2, bf2, Wh1, bh1, Wh2, bh2, Wa, ba, W2h, b2h):
    bf = ml_dtypes.bfloat16
    xh = np.ascontiguousarray(np.asarray(h_out, np.float32).T).astype(bf)
    xf = np.ascontiguousarray(np.asarray(fake_region, np.float32).T).astype(bf)
    cf = np.asarray(conv_feat, np.float32).astype(bf)
    cfe = np.asarray(conv_feat_embed, np.float32).astype(bf)
    shared = {
        "wf1t": np.ascontiguousarray(np.asarray(Wf1, np.float32).T).astype(bf),
        "wf2t": np.ascontiguousarray(np.asarray(Wf2, np.float32).T).astype(bf),
        "wh1t": np.ascontiguousarray(np.asarray(Wh1, np.float32).T).astype(bf),
        "wh2t": np.ascontiguousarray(np.asarray(Wh2, np.float32).T).astype(bf),
        "w2ht": np.ascontiguousarray(np.asarray(W2h, np.float32).T).astype(bf),
        "bf1": np.asarray(bf1, np.float32).reshape(1, R).astype(bf),
        "bf2": np.asarray(bf2, np.float32).reshape(1, R).astype(bf),
        "bh1": np.asarray(bh1, np.float32).reshape(1, R).astype(bf),
        "bh2": np.asarray(bh2, np.float32).reshape(1, R).astype(bf),
        "b2h": np.asarray(b2h, np.float32).reshape(1, R).astype(bf),
        "wa": np.asarray(Wa, np.float32).reshape(1, A).astype(bf),
    }
    in_maps = []
    for i in range(N_CORES):
        s = slice(i * BS, (i + 1) * BS)
        m = dict(shared)
        m["xh"] = np.ascontiguousarray(xh[:, s])
        m["xf"] = np.ascontiguousarray(xf[:, s])
        m["cf"] = np.ascontiguousarray(cf[s])
        m["cfe"] = np.ascontiguousarray(cfe[s])
        in_maps.append(m)
    return in_maps


def kernel(**inputs):
    nc = _get_nc()
    in_maps = _prep_inputs(**inputs)
    res = run_bass_kernel_spmd(nc, in_maps, core_ids=list(range(N_CORES)))
    return np.concatenate([res.results[i]["out"] for i in range(N_CORES)], axis=0)


def run_traced(**inputs):
    """Like kernel() but with NTFF tracing; returns (out, BassKernelResults)."""
    nc = _get_nc()
    in_maps = _prep_inputs(**inputs)
    res = run_bass_kernel_spmd(nc, in_maps, core_ids=list(range(N_CORES)),
                               trace=True)
    out = np.concatenate([res.results[i]["out"] for i in range(N_CORES)], axis=0)
    return out, res
